# revision 17
# baseline (speedup 1.0000x reference)
"""Trainium2 Bass kernel for nn_DifferentiableBundleAdjustment.

Reference semantics (B=4096, S=512, STATE_DIM=15):
    delta = dba_params[..., :7] * 0.1
    init  = gt_state[:, 0, :7]
    p_s = p_{s-1} + delta_p[s-1]                 (channels 0:3, prefix sum)
    q_s = normalize(q_{s-1} + delta_q[s-1])      (channels 3:7, serial scan)
    out[..., :7] = states, out[..., 7:15] = 0

Strategy: pure batch data-parallel over 8 cores (512 trajectories/core,
128 partitions x 4 groups).  The 511-step serial quaternion scan runs as
THREE dependent custom-DVE instructions per step, all on the Vector
engine (same-engine program order, no semaphores on the critical cycle):

  i1 AXPY : u  = q_prev + 0.1*d_raw            (fused prescale)
  i2 WINSQ: ss = per-4-window reset cumsum of u^2   (hand-built 3-state
            uop FSM using the SUB_DIM_DONE trigger; window ends hold
            ||u_g||^2)
  i3 NRMUL: q  = u * F(ss_bcast), F = y0*(C2 - ss*y0^2),
            y0 = C0 + C1*bitcast(~ss)  -- an 8-stage bit-trick rsqrt
            seed polished by one scaled Newton step, constants fitted
            against the empirical ss distribution (rms 0.09%, end-to-end
            chain error 0.037 abs vs 0.27 tolerance).

Step 1 (unnormalized gt seed, ss up to 19) uses an exact ACT-engine
rsqrt instead of F.  Positions use tensor_tensor_scan; output rows are
assembled in SBUF and written with large contiguous DMAs.
"""

import numpy as np
from contextlib import ExitStack

import concourse.bass as bass
import concourse.tile as tile
from concourse import mybir
from concourse.bass_utils import run_bass_kernel_spmd

# ----------------------------------------------------------------------------
# Problem constants (hardcoded per harness contract)
# ----------------------------------------------------------------------------
B_FULL = 4096
S_FULL = 512
P_DBA = 32
STATE_DIM = 15
N_CORES = 8
B_SHARD = B_FULL // N_CORES        # 512 trajectories per core
P = 128                            # SBUF partitions
G = B_SHARD // P                   # 4 trajectory groups per core

# rsqrt approx F(z) = y0*(C2 - z*y0^2), y0 = C0 + C1*bitcast(~z); constants
# fitted (weighted by the empirical z distribution over steps 2..511, with a
# 2% cap on [0.24, 2.36]) -- rms rel err 0.09%, max 2.4%.
NR_C0 = 0.42547471419508776
NR_C1 = -0.08378698031665198
NR_C2 = 1.8908473805755992

_REGISTERED = {}
_PATCHED = {}


def _split_multiwait_json(bir_json: bytes) -> bytes:
    """This walrus build accepts only one sync-wait command per instruction.
    Tile emits joins with several waits; split the extras onto single-wait
    NoOps inserted just before (engines execute in order, so blocking the
    engine on a preceding NoOp is equivalent).

    NOTE: eliding same-engine DVE->DVE semaphore waits (to save the ~35ns
    round-trip per chain instruction) was tried and hung the device
    (NRT_EXEC_UNIT_UNRECOVERABLE) -- the DVE pipeline does NOT interlock
    same-engine RAW hazards; the semaphores are load-bearing."""
    import json
    d = json.loads(bir_json)
    ctr = 0
    changed_any = False
    for fn in d.get("functions", []):
        for blk in fn.get("blocks", []):
            insts = blk.get("instructions", [])
            out = []
            changed = False
            for ins in insts:
                si = ins.get("sync_info") or {}
                waits = si.get("on_wait") or []
                if len(waits) > 1:
                    for w in waits[:-1]:
                        ctr += 1
                        out.append({
                            "debug": ins.get("debug", 0),
                            "engine": ins["engine"],
                            "ins": [],
                            "outs": [],
                            "name": f"{ins['name']}-mw{ctr}",
                            "opcode": "NoOp",
                            "sync_info": {"on_wait": [w]},
                        })
                    si["on_wait"] = [waits[-1]]
                    changed = True
                out.append(ins)
            if changed:
                blk["instructions"] = out
                changed_any = True
    if not changed_any:
        return bir_json
    return json.dumps(d).encode()


def _install_compile_patch():
    """Route every compile_bir_kernel call through the multi-wait splitter."""
    if _PATCHED:
        return
    import concourse.bass_utils as bu
    orig = bu.compile_bir_kernel

    def patched(bir_json, tmpdir, neff_name="file.neff"):
        return orig(_split_multiwait_json(bytes(bir_json)), tmpdir,
                    neff_name=neff_name)

    bu.compile_bir_kernel = patched
    try:
        import concourse.bass2jax as b2j
        b2j.compile_bir_kernel = patched
    except Exception:
        pass
    _PATCHED["on"] = True


def _register_ops():
    """Register the custom DVE ops (runtime, idempotent)."""
    if _REGISTERED:
        return _REGISTERED
    import concourse.dve_spec as DS
    import concourse.dve_ops as dve_ops
    from concourse.dve_spec import (
        Spec, Src0, Src1, C0, C1, C2, AluOp, Bin, lower, sq, scan, _has_src1,
    )
    from concourse.dve_uop import DveOpSpec, Trigger

    def add_op(name, spec, subdim, uops_by_ver):
        if name in dve_ops._SUB_OPCODE_FOR_NAME:
            _REGISTERED[name] = next(o for o in dve_ops.OPS if o.name == name)
            return
        shas = {}
        for ver, uops in uops_by_ver.items():
            shas[ver] = DveOpSpec(name=name, opcode=1, uops=uops,
                                  rd1_en=_has_src1(spec)).sha(ver)
        op = dve_ops.DveOp(name, spec, subdim=subdim, uops_sha=shas)
        dve_ops.OPS.append(op)
        row = dve_ops._CUSTOM_DVE_ROW_BASE + len(dve_ops.OPS) - 1
        dve_ops._SUB_OPCODE_FOR_NAME[name] = row
        dve_ops.CUSTOM_DVE_SPECS[name] = op.spec
        for ver, uops in uops_by_ver.items():
            dve_ops._COMPILE_CACHE[(name, ver)] = DveOpSpec(
                name=name, opcode=row, uops=uops, rd1_en=_has_src1(spec))
        _REGISTERED[name] = op

    # --- AXPY: out = Src0 + C0*Src1 ---
    axpy_spec = Spec(
        body=Src0 + C0 * Src1,
        reference=lambda in0, in1, s0, s1, imm2: (
            np.asarray(in0, np.float32) + np.float32(s0) *
            np.asarray(in1, np.float32)).astype(np.float32),
    )
    add_op("ANT_DBA2_AXPY", axpy_spec, False,
           {v: lower(axpy_spec, ver=v) for v in ("v3", "v4")})

    # --- WINSQ: per-subdim-row reset cumsum of squares (hand-built FSM) ---
    def _winsq_ref(in0, s0, s1, imm2):
        a = np.asarray(in0, np.float32)
        sqv = (a * a).astype(np.float32)
        return np.cumsum(sqv, axis=-1, dtype=np.float32)

    def build_winsq(ver):
        spec = Spec(body=scan(AluOp.ADD, sq(Src0)), reference=_winsq_ref)
        DS._validate_body(spec, ver)
        spec2 = DS._hoist_stream_invariant_ops(spec)
        scans = DS._collect(spec2.body, DS.Scan)
        placement = DS._build_placement(
            spec2, scans, DS.N_STAGES[ver], DS.N_LANES[ver])
        seed_ov, step_ov = DS._scan_overrides(scans, placement.node_stage)
        assert not step_ov
        sc = scans[0]
        d = placement.node_stage[sc]
        states = [
            DS._State(placement=placement, overrides=seed_ov,
                      trigger=DS.COUNT_ONCE, repeat=1, next=(1, 0, 0),
                      write_out=False),
            DS._State(placement=placement, consume=(True, False),
                      trigger=(Trigger.SRC_TENSOR_DONE, Trigger.SUB_DIM_DONE,
                               Trigger.NONE),
                      next=(0, 2, 0)),
            DS._State(placement=placement, consume=(True, False),
                      overrides={d: DS._Stage(AluOp.ADD, DS.Zero, sc.expr)},
                      trigger=(Trigger.SRC_TENSOR_DONE, Trigger.SUB_DIM_DONE,
                               Trigger.COUNT),
                      next=(0, 2, 1), repeat=1),
        ]
        uops = [DS._assemble(s) for s in states]
        for u in uops:
            u.validate(ver)
        return spec, uops

    winsq_spec, uops3 = build_winsq("v3")
    _, uops4 = build_winsq("v4")
    add_op("ANT_DBA2_WINSQ", winsq_spec, True, {"v3": uops3, "v4": uops4})

    # --- NRMUL: out = Src1*(y0*(C1 - Src0*y0^2)), y0 = C0 + C2*~Src0 ---
    # wiring: s0 = NR_C0, s1 = NR_C2, imm2 = NR_C1
    _nz = Bin(AluOp.BITWISE_NOT, Src0, Src0)
    _y0 = C0 + C2 * _nz

    def _nrmul_ref(in0, in1, s0, s1, imm2):
        z = np.ascontiguousarray(np.asarray(in0, np.float32))
        nzv = (~z.view(np.int32)).view(np.float32)
        y0 = (np.float32(s0) + np.float32(imm2) * nzv).astype(np.float32)
        e = (np.float32(s1) - z * (y0 * y0).astype(np.float32)
             ).astype(np.float32)
        return (np.asarray(in1, np.float32) * (y0 * e).astype(np.float32)
                ).astype(np.float32)

    nrmul_spec = Spec(
        body=Src1 * (_y0 * (C1 - Src0 * (_y0 * _y0))),
        reference=_nrmul_ref,
    )
    add_op("ANT_DBA2_NRMUL", nrmul_spec, False,
           {v: lower(nrmul_spec, ver=v) for v in ("v3", "v4")})
    return _REGISTERED


# ----------------------------------------------------------------------------
# Bass module builder (one core's program; SPMD across cores via in_maps)
# ----------------------------------------------------------------------------

def build_nc(S=S_FULL, CS=64, b_shard=B_SHARD):
    ops = _register_ops()
    _install_compile_patch()
    g = b_shard // P
    assert g * P == b_shard
    SD = S - 1                       # number of delta steps used
    nchunk = (SD + CS - 1) // CS

    f32 = mybir.dt.float32
    nc = bass.Bass()
    dba7 = nc.dram_tensor("dba7", [b_shard, S, 7], f32, kind="ExternalInput")
    gt7 = nc.dram_tensor("gt7", [b_shard, 7], f32, kind="ExternalInput")
    out = nc.dram_tensor("out", [b_shard, S, STATE_DIM], f32,
                         kind="ExternalOutput")

    TRAJ_STRIDE = S * 7              # dba7 elements per trajectory
    OUT_TRAJ = S * STATE_DIM

    with ExitStack() as ctx:
        tc = ctx.enter_context(tile.TileContext(nc))
        persist = ctx.enter_context(tc.tile_pool(name="persist", bufs=1))
        raw_pool = ctx.enter_context(tc.tile_pool(name="raw", bufs=3))

        u_t = persist.tile([P, 4 * g], f32, tag="u")
        ss_t = persist.tile([P, 4 * g], f32, tag="ss")
        rn16_t = persist.tile([P, 4 * g], f32, tag="rn16")
        gtin_t = persist.tile([P, 7 * g], f32, tag="gtin")
        c01_t = persist.tile([P, 1], f32, tag="c01")
        ones_t = persist.tile([P, SD], f32, tag="ones")
        iout_t = persist.tile([P, STATE_DIM * g], f32, tag="iout")
        # whole-sequence tiles: staging for all 511 output rows (group-major)
        # and the packed prescaled position deltas
        stg_t = persist.tile([P, g * SD * STATE_DIM], f32, tag="stg")
        del3_t = persist.tile([P, g * SD * 3], f32, tag="del3")
        G_STG = SD * STATE_DIM           # stg elements per group
        G_D3 = SD * 3                    # del3 elements per group

        def ap(t, off, dims):
            return bass.AP(t.tensor, t[:].offset + off,
                           [t[:].ap[0]] + list(dims))

        # gt init load: single DMA covering all trajectory groups
        nc.sync.dma_start(
            ap(gtin_t, 0, [[7, g], [1, 7]]),
            bass.AP(gt7, 0, [[7, P], [P * 7, g], [1, 7]]),
        )

        def act_rsqrt(out_ap, in_ap):
            # exact rsqrt on the Scalar engine (used only for step 1, whose
            # unnormalized gt seed puts ss in [0.09, 19.2], outside the
            # fitted range of the in-loop approximation)
            eng = nc.scalar
            bias_ap = nc.const_aps.scalar_like(0.0, in_ap)
            eng.add_instruction(mybir.InstActivation(
                name=nc.get_next_instruction_name(),
                func=mybir.ActivationFunctionType.Rsqrt,
                ins=[eng.lower_ap(in_ap), eng.lower_ap(bias_ap),
                     mybir.ImmediateValue(dtype=mybir.dt.float32, value=1.0),
                     mybir.ImmediateValue(dtype=mybir.dt.float32, value=0.0)],
                outs=[eng.lower_ap(out_ap)]))

        def fill_const(dst_ap, val):
            nc.gpsimd.memset(dst_ap, float(val))

        fill_const(ones_t[:], 1.0)
        fill_const(iout_t[:], 0.0)
        fill_const(c01_t[:], 0.1)

        # s=0 output row: channels 0:7 = gt init, rest zero
        nc.gpsimd.tensor_copy(
            ap(iout_t, 0, [[STATE_DIM, g], [1, 7]]),
            ap(gtin_t, 0, [[7, g], [1, 7]]),
        )
        nc.sync.dma_start(
            bass.AP(out, 0, [[OUT_TRAJ, P], [P * OUT_TRAJ, g],
                             [1, STATE_DIM]]),
            ap(iout_t, 0, [[STATE_DIM, g], [1, STATE_DIM]]),
        )

        AXPY = ops["ANT_DBA2_AXPY"]
        WINSQ = ops["ANT_DBA2_WINSQ"]
        NRMUL = ops["ANT_DBA2_NRMUL"]

        def emit_pos_scans():
            # whole-sequence position prefix sums (12 vector instrs); emitted
            # in vector program order well after the gpsimd prescales finish
            for gi in range(g):
                for c in range(3):
                    nc.vector.tensor_tensor_scan(
                        ap(stg_t, gi * G_STG + c, [[STATE_DIM, SD]]),
                        ap(ones_t, 0, [[1, SD]]),
                        ap(del3_t, gi * G_D3 + c, [[3, SD]]),
                        ap(gtin_t, gi * 7 + c, [[1, 1]]),
                        mybir.AluOpType.mult,
                        mybir.AluOpType.add,
                    )

        for k in range(nchunk):
            nk = min(CS, SD - k * CS)
            raw_t = raw_pool.tile([P, g * CS * 7], f32, tag="raw")

            # zero this chunk's staging rows (split per chunk so the chain's
            # first writes wait ~3us for piece 0, not 27us for the whole tile)
            fill_const(ap(stg_t, k * CS * STATE_DIM,
                          [[G_STG, g], [1, nk * STATE_DIM]]), 0.0)

            # load chunk deltas (contiguous per trajectory), one DMA for all;
            # chunk 0 is split so the first steps' data lands ~5us earlier
            # (the chain start is gated by this transfer)
            pieces = [(0, 8), (8, nk - 8)] if k == 0 else [(0, nk)]
            for off, n in pieces:
                nc.sync.dma_start(
                    ap(raw_t, off * 7, [[CS * 7, g], [1, n * 7]]),
                    bass.AP(dba7, (k * CS + off) * 7,
                            [[TRAJ_STRIDE, P], [P * TRAJ_STRIDE, g],
                             [1, n * 7]]),
                )
            # pack+prescale position deltas (gpsimd, off critical path)
            for gi in range(g):
                nc.gpsimd.tensor_mul(
                    ap(del3_t, gi * G_D3 + k * CS * 3, [[3, nk], [1, 3]]),
                    ap(raw_t, gi * CS * 7, [[7, nk], [1, 3]]),
                    ap(c01_t, 0, [[0, nk], [0, 3]]),
                )

            # whole-sequence position scans, emitted once every prescale is
            # in program order (gpsimd finishes them by ~31us; the vector
            # chain reaches this point at ~280us, so no stall).  The output
            # DMAs of all earlier chunks read the position channels, so they
            # are deferred to right after the scans.
            if k == nchunk - 1:
                emit_pos_scans()
                for kk in range(nchunk - 1):
                    nkk = min(CS, SD - kk * CS)
                    nc.sync.dma_start(
                        bass.AP(out, (kk * CS + 1) * STATE_DIM,
                                [[OUT_TRAJ, P], [P * OUT_TRAJ, g],
                                 [1, nkk * STATE_DIM]]),
                        ap(stg_t, kk * CS * STATE_DIM,
                           [[G_STG, g], [1, nkk * STATE_DIM]]),
                    )

            # --- quaternion chain: 3 dependent vector instrs per step ---
            for j in range(1, nk + 1):
                row = k * CS + j - 1          # stg row index (global step-1)
                if row == 0:
                    q_ap = ap(gtin_t, 3, [[7, g], [1, 4]])
                else:
                    q_ap = ap(stg_t, (row - 1) * STATE_DIM + 3,
                              [[G_STG, g], [1, 4]])
                d_ap = ap(raw_t, (j - 1) * 7 + 3, [[CS * 7, g], [1, 4]])
                out_ap = ap(stg_t, row * STATE_DIM + 3,
                            [[G_STG, g], [1, 4]])

                nc.vector._custom_dve(
                    AXPY, out=u_t[:], in0=q_ap, in1=d_ap, s0=0.1)
                nc.vector._custom_dve(
                    WINSQ,
                    out=ap(ss_t, 0, [[4, g], [1, 4]]),
                    in0=ap(u_t, 0, [[4, g], [1, 4]]))
                if row == 0:
                    act_rsqrt(rn16_t[:], ap(ss_t, 3, [[4, g], [0, 4]]))
                    nc.vector.tensor_mul(out_ap,
                                         ap(u_t, 0, [[4, g], [1, 4]]),
                                         ap(rn16_t, 0, [[4, g], [1, 4]]))
                else:
                    nc.vector._custom_dve(
                        NRMUL, out=out_ap,
                        in0=ap(ss_t, 3, [[4, g], [0, 4]]),
                        in1=u_t[:],
                        s0=NR_C0, s1=NR_C2, imm2=NR_C1)

            # drain the final chunk to DRAM in 4 pieces so the DMA tail
            # overlaps the last rows' compute (earlier chunks were deferred
            # above, after the position scans)
            if k == nchunk - 1:
                pc = (nk + 3) // 4
                off = 0
                while off < nk:
                    n = min(pc, nk - off)
                    nc.sync.dma_start(
                        bass.AP(out, (k * CS + off + 1) * STATE_DIM,
                                [[OUT_TRAJ, P], [P * OUT_TRAJ, g],
                                 [1, n * STATE_DIM]]),
                        ap(stg_t, (k * CS + off) * STATE_DIM,
                           [[G_STG, g], [1, n * STATE_DIM]]),
                    )
                    off += n

    from concourse.library_overlay import lower_extended_insts
    lower_extended_insts(nc)
    return nc


# ----------------------------------------------------------------------------
# Host entry point
# ----------------------------------------------------------------------------
_NC_CACHE = {}


def _get_nc():
    if "nc" not in _NC_CACHE:
        _NC_CACHE["nc"] = build_nc()
    return _NC_CACHE["nc"]


def kernel(dba_params, imu_measurements=None, gt_state=None, **_unused):
    dba_params = np.asarray(dba_params, dtype=np.float32)
    gt_state = np.asarray(gt_state, dtype=np.float32)
    assert dba_params.shape == (B_FULL, S_FULL, P_DBA)
    dba7 = np.ascontiguousarray(dba_params[:, :, :7])
    gt7 = np.ascontiguousarray(gt_state[:, 0, :7])

    nc = _get_nc()
    in_maps = [
        {"dba7": dba7[i * B_SHARD:(i + 1) * B_SHARD],
         "gt7": gt7[i * B_SHARD:(i + 1) * B_SHARD]}
        for i in range(N_CORES)
    ]
    res = run_bass_kernel_spmd(nc, in_maps, core_ids=list(range(N_CORES)))
    return np.concatenate([res.results[i]["out"] for i in range(N_CORES)],
                          axis=0)
